# revision 5
# baseline (speedup 1.0000x reference)
"""MoE transformer (B=2,S=1024,D=768,H=12,F=3072,E=8,K=2,V=32000,L=2) on 8 TRN2 cores.

Sharding: expert-parallel (core c owns expert c) + vocab-parallel LM head
(core c computes logits for vocab [c*4000,(c+1)*4000)). Attention/embedding
replicated. One AllReduce of the cw-scaled expert output per layer.

Precision: fp32 for everything feeding the router logits (QKV/WO/gate);
float32r (TF32-class, full PE speed) for attention scores/PV, both FFNs and
the LM head. This keeps top-2 expert selection identical to the fp32
reference (min routing margin 2.6e-5 >> f32r-path noise) while running the
FLOP-heavy parts at ~4.6x the fp32 matmul rate.

Layout: residual stream feature-major [768(part), 2048(tok)], token
t = c*128 + p. Attention computed transposed (S_T[k,q]); softmax denominator
comes from an all-ones-lhsT matmul that broadcasts the column sums to all
PSUM partitions; causal blocks above the diagonal are skipped. Heads are
paired in 128-row tiles: even head values in v cols 0:64, odd in 64:128, so
PV output lands at the partition base its head needs.
"""
import sys

sys.path.insert(0, "/opt/trn_rl_repo")

import numpy as np

import concourse.bass as bass
import concourse.mybir as mybir
import concourse.tile as tile
from concourse import bacc
from concourse.bass_utils import run_bass_kernel_spmd
from concourse.masks import make_identity, make_upper_triangular

B, S, D, H, F, E, TOPK, VOC, L = 2, 1024, 768, 12, 3072, 8, 2, 32000, 2
T = B * S
P = 128
HD = D // H               # 64
DT = D // P               # 6
TT = T // P               # 16
ST = S // P               # 8
CORES = 8
VS = VOC // CORES         # 4000
VCH = 500
NVCH = VS // VCH          # 8
CH = 512                  # token chunk
NCH = T // CH             # 4 global chunks
FB = 256                  # ff band
NB = F // FB              # 12 bands
FT = FB // P              # 2
EPS = 1e-6

F32 = mybir.dt.float32
F32R = mybir.dt.float32r
I32 = mybir.dt.int32
AF = mybir.ActivationFunctionType
ALU = mybir.AluOpType
AX = mybir.AxisListType

_compiled = None


def fm(ap):
    """[(o p), rest...] -> [p, o, rest...] with p=128 (feature-major load)."""
    letters = "abcdefg"[: len(ap.shape) - 1]
    src = "(o p) " + " ".join(letters)
    dst = "p o " + " ".join(letters)
    return ap.rearrange(f"{src} -> {dst}", p=P)


def build():
    nc = bacc.Bacc(None, target_bir_lowering=False, debug=False)

    tok_d = nc.dram_tensor("tok32", [T], I32, kind="ExternalInput")
    emb_d = nc.dram_tensor("emb", [VOC, D], F32, kind="ExternalInput")
    pos_d = nc.dram_tensor("pos", [S, D], F32, kind="ExternalInput")
    wq_d = nc.dram_tensor("wq", [L, D, D], F32, kind="ExternalInput")   # ln1-folded, /8
    wk_d = nc.dram_tensor("wk", [L, D, D], F32, kind="ExternalInput")   # ln1-folded
    wv_d = nc.dram_tensor("wv", [L, D, D], F32, kind="ExternalInput")   # ln1-folded
    wo_d = nc.dram_tensor("wo", [L, D, D], F32, kind="ExternalInput")
    bq_d = nc.dram_tensor("bq", [L, D], F32, kind="ExternalInput")      # /8
    bk_d = nc.dram_tensor("bk", [L, D], F32, kind="ExternalInput")
    bv_d = nc.dram_tensor("bv", [L, D], F32, kind="ExternalInput")
    bo_d = nc.dram_tensor("bo", [L, D], F32, kind="ExternalInput")
    gw_d = nc.dram_tensor("gw", [L, D, E], F32, kind="ExternalInput")   # ln2-folded
    gb_d = nc.dram_tensor("gb", [L, E], F32, kind="ExternalInput")
    w1_d = nc.dram_tensor("w1e", [L, D, F], F32R, kind="ExternalInput")  # ln2-folded
    b1_d = nc.dram_tensor("b1e", [L, F], F32, kind="ExternalInput")
    w2_d = nc.dram_tensor("w2e", [L, F, D], F32R, kind="ExternalInput")
    b2_d = nc.dram_tensor("b2e", [L, D], F32, kind="ExternalInput")
    oh_d = nc.dram_tensor("onehot", [1, E], F32, kind="ExternalInput")
    eh_d = nc.dram_tensor("embT", [D, VS], F32R, kind="ExternalInput")  # lnf-folded
    out_d = nc.dram_tensor("out", [T, VS], F32, kind="ExternalOutput")

    with tile.TileContext(nc) as tc, \
         tc.tile_pool(name="sb", bufs=1) as sb, \
         tc.tile_pool(name="pa", bufs=2, space="PSUM") as pa, \
         tc.tile_pool(name="ob", bufs=6, space="PSUM") as ob, \
         tc.tile_pool(name="dram", bufs=1, space="DRAM") as dram:

        # ---- constants ----
        ident = sb.tile([P, P], F32)
        make_identity(nc, ident)
        tri = sb.tile([P, P], F32)   # additive causal mask in [k,q]: 0 if q>=k else -1e9
        make_upper_triangular(nc, tri, val=1.0, diag=True)
        nc.vector.tensor_scalar(tri[:], tri[:], 1.0, 1e9, ALU.subtract, ALU.mult)
        onesf = sb.tile([P, P], F32)
        nc.gpsimd.memset(onesf[:], 1.0)
        ones_r = sb.tile([P, P], F32R)
        nc.vector.tensor_copy(ones_r[:], onesf[:])
        eps_t = sb.tile([P, 1], F32)
        nc.gpsimd.memset(eps_t[:], EPS)

        # ---- big persistent / overlaid tiles ----
        x_fm = sb.tile([P, DT, T], F32, tag="x")

        def norm_bc(t0, width, tag="nbc"):
            """rstd broadcast tile for x tokens [t0,t0+width)."""
            pr = pa.tile([P, width], F32, tag="pa", name="prn")
            for k in range(DT):
                sq = sb.tile([P, width], F32R, tag="probs", bufs=2, name="sq")
                nc.scalar.activation(sq[:], x_fm[:, k, t0:t0 + width], AF.Square)
                nc.tensor.matmul(pr[:1, :], ones_r[:, :1], sq[:],
                                 start=(k == 0), stop=(k == DT - 1))
            srow = sb.tile([1, width], F32, tag="srow", bufs=1, name="srow")
            nc.scalar.activation(srow[:1, :], pr[:1, :], AF.Sqrt, bias=eps_t[:1, :], scale=1.0 / D)
            rrow = sb.tile([1, width], F32, tag="rrow", bufs=1, name="rrow")
            nc.vector.reciprocal(rrow[:1, :], srow[:1, :])
            bc = sb.tile([P, width], F32, tag=tag, bufs=2, name="bc")
            nc.gpsimd.partition_broadcast(bc[:], rrow[:1, :])
            return bc

        # ================= embedding =================
        with nc.named_scope("embed"):
            tok_s = sb.tile([P, TT], I32, tag="toks")
            nc.sync.dma_start(tok_s[:], tok_d.ap().rearrange("(c p) -> p c", p=P))
            pos_s = sb.tile([P, ST, D], F32, tag="vv", name="pos_s")
            nc.sync.dma_start(pos_s[:], pos_d.ap().rearrange("(c p) d -> p c d", p=P))
            for b in range(B):
                x_tm = sb.tile([P, ST, D], F32, tag="kb", name="x_tm")
                for c in range(ST):
                    nc.gpsimd.indirect_dma_start(
                        out=x_tm[:, c, :], out_offset=None,
                        in_=emb_d.ap(),
                        in_offset=bass.IndirectOffsetOnAxis(
                            ap=tok_s[:, b * ST + c:b * ST + c + 1], axis=0),
                    )
                nc.vector.tensor_add(x_tm[:], x_tm[:], pos_s[:])
                for c in range(ST):
                    for k in range(DT):
                        tp = pa.tile([P, P], F32, tag="pa", name="tp")
                        nc.tensor.transpose(tp[:], x_tm[:, c, k * P:(k + 1) * P], ident[:])
                        nc.vector.tensor_copy(
                            x_fm[:, k, b * S + c * P:b * S + (c + 1) * P], tp[:])

        # ================= layers =================
        for l in range(L):
            # ---------- attention ----------
            bq_s = sb.tile([P, DT], F32, tag="bq", name="bq_s")
            bk_s = sb.tile([P, DT], F32, tag="bk", name="bk_s")
            bo_s = sb.tile([P, DT], F32, tag="bo", name="bo_s")
            nc.sync.dma_start(bq_s[:], fm(bq_d.ap()[l]))
            nc.sync.dma_start(bk_s[:], fm(bk_d.ap()[l]))
            nc.sync.dma_start(bo_s[:], fm(bo_d.ap()[l]))
            bvrow = sb.tile([1, D], F32, tag="bvrow", name="bvrow")
            nc.sync.dma_start(bvrow[:1, :], bv_d.ap()[l][None, :])
            bv_bc = sb.tile([P, D], F32, tag="bvbc", name="bv_bc")
            nc.gpsimd.partition_broadcast(bv_bc[:], bvrow[:1, :])

            with nc.named_scope(f"attn_{l}"):
                for b in range(B):
                    k_b = sb.tile([P, DT, S], F32R, tag="kb", name="k_b")
                    v_b = sb.tile([P, ST, DT, P], F32R, tag="vv", name="v_b")
                    for ci in range(2):  # 512-token chunks within batch
                        t0 = b * S + ci * CH
                        bc = norm_bc(t0, CH)
                        hn_c = sb.tile([P, DT, CH], F32, tag="normo", name="hn_c")
                        for k in range(DT):
                            nc.vector.tensor_tensor(hn_c[:, k], x_fm[:, k, t0:t0 + CH],
                                                    bc[:], ALU.mult)
                        # q/k projections (fp32) -> f32r
                        q_c = sb.tile([P, DT, CH], F32R, tag="qc", name="q_c")
                        for m in range(DT):
                            wts = sb.tile([P, DT, P], F32, tag="wqk", bufs=2, name="wts")
                            nc.sync.dma_start(wts[:], fm(wq_d.ap()[l, :, m * P:(m + 1) * P]))
                            pt = pa.tile([P, CH], F32, tag="pa", name="pq")
                            for k in range(DT):
                                nc.tensor.matmul(pt[:], wts[:, k], hn_c[:, k],
                                                 start=(k == 0), stop=(k == DT - 1))
                            nc.vector.tensor_scalar(q_c[:, m], pt[:], bq_s[:, m:m + 1],
                                                    None, ALU.add)
                            wts2 = sb.tile([P, DT, P], F32, tag="wqk", bufs=2, name="wts2")
                            nc.sync.dma_start(wts2[:], fm(wk_d.ap()[l, :, m * P:(m + 1) * P]))
                            pt2 = pa.tile([P, CH], F32, tag="pa", name="pk")
                            for k in range(DT):
                                nc.tensor.matmul(pt2[:], wts2[:, k], hn_c[:, k],
                                                 start=(k == 0), stop=(k == DT - 1))
                            nc.vector.tensor_scalar(k_b[:, m, ci * CH:(ci + 1) * CH], pt2[:],
                                                    bk_s[:, m:m + 1], None, ALU.add)
                        # v projection (fp32, token-major into head-pair tiles)
                        for j in range(4):  # token tiles in this chunk
                            tt = ci * 4 + j
                            for m in range(DT):  # wv col tiles of 128 = head pair
                                wts3 = sb.tile([P, DT, P], F32, tag="wqk", bufs=2, name="wts3")
                                nc.sync.dma_start(wts3[:], fm(wv_d.ap()[l, :, m * P:(m + 1) * P]))
                                pt3 = pa.tile([P, P], F32, tag="pa", name="pv")
                                for k in range(DT):
                                    nc.tensor.matmul(pt3[:], hn_c[:, k, j * P:(j + 1) * P],
                                                     wts3[:, k],
                                                     start=(k == 0), stop=(k == DT - 1))
                                nc.vector.tensor_tensor(v_b[:, tt, m, :], pt3[:],
                                                        bv_bc[:, m * P:(m + 1) * P], ALU.add)
                        # attention core (f32r)
                        nkt = 4 * ci + 4
                        att_c = sb.tile([P, DT, CH], F32, tag="attoc", name="att_c")
                        for h in range(H):
                            par, mt = h % 2, h // 2
                            r0 = par * HD
                            pv = ob.tile([P, CH], F32, tag="ob", name="pvp")
                            den = ob.tile([P, CH], F32, tag="ob", name="denp")
                            for kt in range(nkt):
                                ps = pa.tile([P, CH], F32, tag="pa", name="psc")
                                nc.tensor.matmul(
                                    ps[:],
                                    k_b[r0:r0 + HD, mt, kt * P:(kt + 1) * P],
                                    q_c[r0:r0 + HD, mt, :],
                                    start=True, stop=True)
                                pr = sb.tile([P, CH], F32R, tag="probs", bufs=2, name="pr")
                                off = (kt - 4 * ci) * P
                                if off >= 0:  # diagonal block
                                    nc.vector.tensor_tensor(ps[:, off:off + P],
                                                            ps[:, off:off + P], tri[:], ALU.add)
                                    if off > 0:
                                        nc.scalar.mul(pr[:, 0:off], ps[:, 0:off], 0.0)
                                    nc.scalar.activation(pr[:, off:], ps[:, off:], AF.Exp)
                                else:
                                    nc.scalar.activation(pr[:], ps[:], AF.Exp)
                                lhs = v_b[:, kt, mt, :] if par else v_b[:, kt, mt, 0:HD]
                                nc.tensor.matmul(pv[:lhs.shape[-1], :], lhs, pr[:],
                                                 start=(kt == 0), stop=(kt == nkt - 1))
                                nc.tensor.matmul(den[:], ones_r[:], pr[:],
                                                 start=(kt == 0), stop=(kt == nkt - 1))
                            rc = sb.tile([P, CH], F32, tag="rc", bufs=2, name="rc")
                            nc.vector.reciprocal(rc[r0:r0 + HD, :], den[r0:r0 + HD, :])
                            nc.vector.tensor_tensor(att_c[r0:r0 + HD, mt, :],
                                                    pv[r0:r0 + HD, :], rc[r0:r0 + HD, :],
                                                    ALU.mult)
                        # WO (fp32) + residual
                        for m in range(DT):
                            wts4 = sb.tile([P, DT, P], F32, tag="wqk", bufs=2, name="wts4")
                            nc.sync.dma_start(wts4[:], fm(wo_d.ap()[l, :, m * P:(m + 1) * P]))
                            pt4 = pa.tile([P, CH], F32, tag="pa", name="pwo")
                            for k in range(DT):
                                nc.tensor.matmul(pt4[:], wts4[:, k], att_c[:, k],
                                                 start=(k == 0), stop=(k == DT - 1))
                            st = sb.tile([P, CH], F32, tag="st", bufs=2, name="st")
                            nc.vector.tensor_scalar(st[:], pt4[:], bo_s[:, m:m + 1],
                                                    None, ALU.add)
                            nc.vector.tensor_add(x_fm[:, m, t0:t0 + CH],
                                                 x_fm[:, m, t0:t0 + CH], st[:])

            # ---------- gate + FFN + AllReduce ----------
            gws = sb.tile([P, DT, E], F32, tag="gws", name="gws")
            nc.sync.dma_start(gws[:], fm(gw_d.ap()[l]))
            gbrow = sb.tile([1, E], F32, tag="gbrow", name="gbrow")
            nc.sync.dma_start(gbrow[:1, :], gb_d.ap()[l][None, :])
            gb_bc = sb.tile([P, E], F32, tag="gbbc", name="gb_bc")
            nc.gpsimd.partition_broadcast(gb_bc[:], gbrow[:1, :])
            ohrow = sb.tile([1, E], F32, tag="ohrow", name="ohrow")
            nc.sync.dma_start(ohrow[:1, :], oh_d.ap())
            oh_bc = sb.tile([P, E], F32, tag="ohbc", name="oh_bc")
            nc.gpsimd.partition_broadcast(oh_bc[:], ohrow[:1, :])
            b1_s = sb.tile([P, F // P], F32, tag="b1s", name="b1_s")
            nc.sync.dma_start(b1_s[:], b1_d.ap()[l].rearrange("(f p) -> p f", p=P))
            b2_s = sb.tile([P, DT], F32, tag="b2s", name="b2_s")
            nc.sync.dma_start(b2_s[:], fm(b2_d.ap()[l]))
            cc_in = dram.tile([D, T], F32, name=f"ccin{l}")
            cc_out = dram.tile([D, T], F32, name=f"ccout{l}", addr_space="Shared")

            with nc.named_scope(f"ffn_{l}"):
                for ci in range(NCH):
                    t0 = ci * CH
                    bc = norm_bc(t0, CH)
                    # gate (fp32) + routing weight for this core's expert
                    hf_c = sb.tile([P, DT, CH], F32, tag="attoc", name="hf_c")
                    hr_c = sb.tile([P, DT, CH], F32R, tag="normo", name="hr_c")
                    for k in range(DT):
                        nc.vector.tensor_tensor(hf_c[:, k], x_fm[:, k, t0:t0 + CH],
                                                bc[:], ALU.mult)
                        nc.vector.tensor_tensor(hr_c[:, k], x_fm[:, k, t0:t0 + CH],
                                                bc[:], ALU.mult)
                    lg = sb.tile([P, 4, E], F32, tag="lg", name="lg")
                    for j in range(4):
                        pg = pa.tile([P, E], F32, tag="pa", name="pg")
                        for k in range(DT):
                            nc.tensor.matmul(pg[:], hf_c[:, k, j * P:(j + 1) * P], gws[:, k],
                                             start=(k == 0), stop=(k == DT - 1))
                        nc.vector.tensor_tensor(lg[:, j], pg[:], gb_bc[:], ALU.add)
                    r1 = sb.tile([P, 4, 1], F32, tag="r1", name="r1")
                    nc.vector.tensor_reduce(r1[:], lg[:], axis=AX.X, op=ALU.max)
                    eq1 = sb.tile([P, 4, E], F32, tag="eq1", name="eq1")
                    nc.vector.tensor_tensor(eq1[:], lg[:], r1[:].to_broadcast([P, 4, E]),
                                            ALU.is_equal)
                    msk = sb.tile([P, 4, E], F32, tag="msk", name="msk")
                    nc.vector.tensor_scalar(msk[:], eq1[:], 1e30, None, ALU.mult)
                    nc.vector.tensor_sub(msk[:], lg[:], msk[:])
                    r2 = sb.tile([P, 4, 1], F32, tag="r2", name="r2")
                    nc.vector.tensor_reduce(r2[:], msk[:], axis=AX.X, op=ALU.max)
                    sig = sb.tile([P, 4, 1], F32, tag="sig", name="sig")
                    nc.vector.tensor_sub(sig[:], r1[:], r2[:])
                    nc.scalar.activation(sig[:], sig[:], AF.Sigmoid)
                    eq2 = sb.tile([P, 4, E], F32, tag="eq2", name="eq2")
                    nc.vector.tensor_tensor(eq2[:], lg[:], r2[:].to_broadcast([P, 4, E]),
                                            ALU.is_equal)
                    nc.vector.tensor_tensor(eq1[:], eq1[:], sig[:].to_broadcast([P, 4, E]),
                                            ALU.mult)
                    nc.vector.tensor_scalar(sig[:], sig[:], -1.0, 1.0, ALU.mult, ALU.add)
                    nc.vector.tensor_tensor(eq2[:], eq2[:], sig[:].to_broadcast([P, 4, E]),
                                            ALU.mult)
                    nc.vector.tensor_add(eq1[:], eq1[:], eq2[:])
                    nc.vector.tensor_tensor(eq1[:], eq1[:],
                                            oh_bc[:, None, :].to_broadcast([P, 4, E]),
                                            ALU.mult)
                    cw_tm = sb.tile([P, 4], F32, tag="cwtm", name="cw_tm")
                    nc.vector.tensor_reduce(cw_tm[:], eq1[:], axis=AX.X, op=ALU.add)
                    cwd = dram.tile([CH], F32, name=f"cwd{l}_{ci}")
                    nc.sync.dma_start(cwd[:].rearrange("(c p) -> p c", p=P), cw_tm[:])
                    cwrow = sb.tile([1, CH], F32, tag="cwrow", bufs=1, name="cwrow")
                    nc.sync.dma_start(cwrow[:1, :], cwd[:][None, :])
                    cw_bc = sb.tile([P, CH], F32, tag="cwbc", bufs=2, name="cw_bc")
                    nc.gpsimd.partition_broadcast(cw_bc[:], cwrow[:1, :])

                    # FFN: PSUM-resident accumulation over all 12 bands
                    outp = [ob.tile([P, CH], F32, tag="ob", name=f"po{d}") for d in range(DT)]
                    for n in range(NB):
                        w1b = sb.tile([P, DT, FB], F32R, tag="w1b", bufs=2, name="w1b")
                        nc.sync.dma_start(w1b[:], fm(w1_d.ap()[l, :, n * FB:(n + 1) * FB]))
                        w2b = sb.tile([P, FT, D], F32R, tag="w2b", bufs=2, name="w2b")
                        nc.sync.dma_start(w2b[:], fm(w2_d.ap()[l, n * FB:(n + 1) * FB, :]))
                        h1 = sb.tile([P, FT, CH], F32R, tag="h1", bufs=2, name="h1")
                        for f in range(FT):
                            ph = pa.tile([P, CH], F32, tag="pa", name="ph1")
                            for k in range(DT):
                                nc.tensor.matmul(ph[:], w1b[:, k, f * P:(f + 1) * P], hr_c[:, k],
                                                 start=(k == 0), stop=(k == DT - 1))
                            nc.scalar.activation(h1[:, f], ph[:], AF.Gelu,
                                                 bias=b1_s[:, n * FT + f:n * FT + f + 1])
                        for d in range(DT):
                            for f in range(FT):
                                nc.tensor.matmul(outp[d][:], w2b[:, f, d * P:(d + 1) * P],
                                                 h1[:, f],
                                                 start=(n == 0 and f == 0),
                                                 stop=(n == NB - 1 and f == FT - 1))
                    for d in range(DT):
                        st = sb.tile([P, CH], F32, tag="st", bufs=2, name="stf")
                        nc.vector.tensor_scalar(st[:], outp[d][:], b2_s[:, d:d + 1],
                                                None, ALU.add)
                        nc.vector.tensor_tensor(st[:], st[:], cw_bc[:], ALU.mult)
                        nc.sync.dma_start(fm(cc_in[:])[:, d, t0:t0 + CH], st[:])

            with nc.named_scope(f"ar_{l}"):
                nc.gpsimd.collective_compute(
                    "AllReduce", ALU.add,
                    replica_groups=[list(range(CORES))],
                    ins=[cc_in[:]], outs=[cc_out[:]],
                )
                for d in range(DT):
                    for ci in range(NCH):
                        st = sb.tile([P, CH], F32, tag="st", bufs=2, name="sta")
                        nc.sync.dma_start(st[:], fm(cc_out[:])[:, d, ci * CH:(ci + 1) * CH])
                        nc.vector.tensor_add(x_fm[:, d, ci * CH:(ci + 1) * CH],
                                             x_fm[:, d, ci * CH:(ci + 1) * CH], st[:])

        # ================= final norm + LM head =================
        with nc.named_scope("head"):
            xf = [sb.tile([P, DT, S], F32R, tag="kb", name="xf_a"),
                  sb.tile([P, DT, S], F32R, tag="vv", name="xf_b")]
            for ci in range(NCH):
                t0 = ci * CH
                bc = norm_bc(t0, CH)
                half, o2 = divmod(t0, S)
                for k in range(DT):
                    nc.vector.tensor_tensor(xf[half][:, k, o2:o2 + CH],
                                            x_fm[:, k, t0:t0 + CH], bc[:], ALU.mult)
            for vc in range(NVCH):
                ehs = sb.tile([P, DT, VCH], F32R, tag=("qc" if vc % 2 == 0 else "attoc"),
                              bufs=1, name="ehs")
                nc.sync.dma_start(ehs[:], fm(eh_d.ap()[:, vc * VCH:(vc + 1) * VCH]))
                for c in range(TT):
                    half, ct = divmod(c, ST)
                    pt = pa.tile([P, VCH], F32, tag="pa", name="phd")
                    for k in range(DT):
                        nc.tensor.matmul(pt[:], xf[half][:, k, ct * P:(ct + 1) * P],
                                         ehs[:, k],
                                         start=(k == 0), stop=(k == DT - 1))
                    st = sb.tile([P, VCH], F32, tag="st", bufs=2, name="sth")
                    nc.vector.tensor_copy(st[:], pt[:])
                    nc.sync.dma_start(
                        out_d.ap().rearrange("(c p) v -> p c v", p=P)[:, c, vc * VCH:(vc + 1) * VCH],
                        st[:])

    nc.compile()
    return nc


def _prep_inputs(inputs):
    gi = {k: np.asarray(v) for k, v in inputs.items()}
    ln1 = gi["ln1"].astype(np.float32)
    ln2 = gi["ln2"].astype(np.float32)
    lnf = gi["ln_f"].astype(np.float32)
    emb = np.ascontiguousarray(gi["emb"], np.float32)
    rs = np.float32(np.sqrt(HD))
    common = {
        "tok32": np.ascontiguousarray(gi["tokens"].reshape(T).astype(np.int32)),
        "emb": emb,
        "pos": np.ascontiguousarray(gi["pos_emb"], np.float32),
        "wq": np.ascontiguousarray(ln1[:, :, None] * gi["wq"] / rs, np.float32),
        "wk": np.ascontiguousarray(ln1[:, :, None] * gi["wk"], np.float32),
        "wv": np.ascontiguousarray(ln1[:, :, None] * gi["wv"], np.float32),
        "wo": np.ascontiguousarray(gi["wo"], np.float32),
        "bq": np.ascontiguousarray(gi["bq"] / rs, np.float32),
        "bk": np.ascontiguousarray(gi["bk"], np.float32),
        "bv": np.ascontiguousarray(gi["bv"], np.float32),
        "bo": np.ascontiguousarray(gi["bo"], np.float32),
        "gw": np.ascontiguousarray(ln2[:, :, None] * gi["gate_w"], np.float32),
        "gb": np.ascontiguousarray(gi["gate_b"], np.float32),
    }
    in_maps = []
    for c in range(CORES):
        onehot = np.zeros((1, E), np.float32)
        onehot[0, c] = 1.0
        m = dict(common)
        m["w1e"] = np.ascontiguousarray(ln2[:, :, None] * gi["w1"][:, c], np.float32)
        m["b1e"] = np.ascontiguousarray(gi["b1"][:, c], np.float32)
        m["w2e"] = np.ascontiguousarray(gi["w2"][:, c], np.float32)
        m["b2e"] = np.ascontiguousarray(gi["b2"][:, c], np.float32)
        m["onehot"] = onehot
        m["embT"] = np.ascontiguousarray(
            (emb[c * VS:(c + 1) * VS, :] * lnf[None, :]).T, np.float32)
        in_maps.append(m)
    return in_maps


def kernel(**inputs):
    global _compiled
    if _compiled is None:
        _compiled = build()
    in_maps = _prep_inputs(inputs)
    res = run_bass_kernel_spmd(_compiled, in_maps, core_ids=list(range(CORES)))
    shards = [res.results[c]["out"].reshape(B, S, VS) for c in range(CORES)]
    return np.concatenate(shards, axis=-1)


if __name__ == "__main__":
    import reference as R
    inputs = R.setup_inputs()
    out = kernel(**{k: np.asarray(v) for k, v in inputs.items()})
    print("kernel out", out.shape, out.dtype)


# revision 6
# speedup vs baseline: 1.0219x; 1.0219x over previous
"""MoE transformer (B=2,S=1024,D=768,H=12,F=3072,E=8,K=2,V=32000,L=2) on 8 TRN2 cores.

Sharding: expert-parallel (core c owns expert c) + vocab-parallel LM head
(core c computes logits for vocab [c*4000,(c+1)*4000)). Attention/embedding
replicated. One AllReduce of the cw-scaled expert output per layer.

Precision: fp32 for everything feeding the router logits (QKV/WO/gate);
float32r (TF32-class, full PE speed) for attention scores/PV, both FFNs and
the LM head. This keeps top-2 expert selection identical to the fp32
reference (min routing margin 2.6e-5 >> f32r-path noise) while running the
FLOP-heavy parts at ~4.6x the fp32 matmul rate.

Layout: residual stream feature-major [768(part), 2048(tok)], token
t = c*128 + p. Attention computed transposed (S_T[k,q]); softmax denominator
comes from an all-ones-lhsT matmul that broadcasts the column sums to all
PSUM partitions; causal blocks above the diagonal are skipped. Heads are
paired in 128-row tiles: even head values in v cols 0:64, odd in 64:128, so
PV output lands at the partition base its head needs.
"""
import sys

sys.path.insert(0, "/opt/trn_rl_repo")

import numpy as np

import concourse.bass as bass
import concourse.mybir as mybir
import concourse.tile as tile
from concourse import bacc
from concourse.bass_utils import run_bass_kernel_spmd
from concourse.masks import make_identity, make_upper_triangular

B, S, D, H, F, E, TOPK, VOC, L = 2, 1024, 768, 12, 3072, 8, 2, 32000, 2
T = B * S
P = 128
HD = D // H               # 64
DT = D // P               # 6
TT = T // P               # 16
ST = S // P               # 8
CORES = 8
VS = VOC // CORES         # 4000
VCH = 500
NVCH = VS // VCH          # 8
CH = 512                  # token chunk
NCH = T // CH             # 4 global chunks
FB = 256                  # ff band
NB = F // FB              # 12 bands
FT = FB // P              # 2
EPS = 1e-6

F32 = mybir.dt.float32
F32R = mybir.dt.float32r
I32 = mybir.dt.int32
AF = mybir.ActivationFunctionType
ALU = mybir.AluOpType
AX = mybir.AxisListType

_compiled = None


def fm(ap):
    """[(o p), rest...] -> [p, o, rest...] with p=128 (feature-major load)."""
    letters = "abcdefg"[: len(ap.shape) - 1]
    src = "(o p) " + " ".join(letters)
    dst = "p o " + " ".join(letters)
    return ap.rearrange(f"{src} -> {dst}", p=P)


def build():
    nc = bacc.Bacc(None, target_bir_lowering=False, debug=False)

    tok_d = nc.dram_tensor("tok32", [T], I32, kind="ExternalInput")
    emb_d = nc.dram_tensor("emb", [VOC, D], F32, kind="ExternalInput")
    pos_d = nc.dram_tensor("pos", [S, D], F32, kind="ExternalInput")
    # weights pre-packed on host into SBUF layouts (contiguous DMA)
    wq_d = nc.dram_tensor("wq", [L, DT, P, DT, P], F32, kind="ExternalInput")
    wk_d = nc.dram_tensor("wk", [L, DT, P, DT, P], F32, kind="ExternalInput")
    wv_d = nc.dram_tensor("wv", [L, DT, P, DT, P], F32, kind="ExternalInput")
    wo_d = nc.dram_tensor("wo", [L, DT, P, DT, P], F32, kind="ExternalInput")
    bq_d = nc.dram_tensor("bq", [L, P, DT], F32, kind="ExternalInput")
    bk_d = nc.dram_tensor("bk", [L, P, DT], F32, kind="ExternalInput")
    bv_d = nc.dram_tensor("bv", [L, D], F32, kind="ExternalInput")
    bo_d = nc.dram_tensor("bo", [L, P, DT], F32, kind="ExternalInput")
    gw_d = nc.dram_tensor("gw", [L, P, DT, E], F32, kind="ExternalInput")
    gb_d = nc.dram_tensor("gb", [L, E], F32, kind="ExternalInput")
    w1_d = nc.dram_tensor("w1e", [L, NB, P, DT, FB], F32R, kind="ExternalInput")
    b1_d = nc.dram_tensor("b1e", [L, P, F // P], F32, kind="ExternalInput")
    w2_d = nc.dram_tensor("w2e", [L, NB, P, FT, D], F32R, kind="ExternalInput")
    b2_d = nc.dram_tensor("b2e", [L, P, DT], F32, kind="ExternalInput")
    oh_d = nc.dram_tensor("onehot", [1, E], F32, kind="ExternalInput")
    eh_d = nc.dram_tensor("embT", [NVCH, P, DT, VCH], F32R, kind="ExternalInput")
    pos2_d = nc.dram_tensor("posp", [P, ST, D], F32, kind="ExternalInput")
    out_d = nc.dram_tensor("out", [T, VS], F32, kind="ExternalOutput")

    with tile.TileContext(nc) as tc, \
         tc.tile_pool(name="sb", bufs=1) as sb, \
         tc.tile_pool(name="pa", bufs=2, space="PSUM") as pa, \
         tc.tile_pool(name="ob", bufs=6, space="PSUM") as ob, \
         tc.tile_pool(name="dram", bufs=1, space="DRAM") as dram:

        # ---- constants ----
        ident = sb.tile([P, P], F32)
        make_identity(nc, ident)
        tri = sb.tile([P, P], F32)   # additive causal mask in [k,q]: 0 if q>=k else -1e9
        make_upper_triangular(nc, tri, val=1.0, diag=True)
        nc.vector.tensor_scalar(tri[:], tri[:], 1.0, 1e9, ALU.subtract, ALU.mult)
        onesf = sb.tile([P, P], F32)
        nc.gpsimd.memset(onesf[:], 1.0)
        ones_r = sb.tile([P, P], F32R)
        nc.vector.tensor_copy(ones_r[:], onesf[:])
        eps_t = sb.tile([P, 1], F32)
        nc.gpsimd.memset(eps_t[:], EPS)

        # ---- big persistent / overlaid tiles ----
        x_fm = sb.tile([P, DT, T], F32, tag="x")

        def norm_bc(t0, width, tag="nbc"):
            """rstd broadcast tile for x tokens [t0,t0+width)."""
            pr = pa.tile([P, width], F32, tag="pa", name="prn")
            for k in range(DT):
                sq = sb.tile([P, width], F32R, tag="probs", bufs=2, name="sq")
                nc.scalar.activation(sq[:], x_fm[:, k, t0:t0 + width], AF.Square)
                nc.tensor.matmul(pr[:1, :], ones_r[:, :1], sq[:],
                                 start=(k == 0), stop=(k == DT - 1))
            srow = sb.tile([1, width], F32, tag="srow", bufs=1, name="srow")
            nc.scalar.activation(srow[:1, :], pr[:1, :], AF.Sqrt, bias=eps_t[:1, :], scale=1.0 / D)
            rrow = sb.tile([1, width], F32, tag="rrow", bufs=1, name="rrow")
            nc.vector.reciprocal(rrow[:1, :], srow[:1, :])
            bc = sb.tile([P, width], F32, tag=tag, bufs=2, name="bc")
            nc.gpsimd.partition_broadcast(bc[:], rrow[:1, :])
            return bc

        # ================= embedding =================
        with nc.named_scope("embed"):
            tok_s = sb.tile([P, TT], I32, tag="toks")
            nc.sync.dma_start(tok_s[:], tok_d.ap().rearrange("(c p) -> p c", p=P))
            pos_s = sb.tile([P, ST, D], F32, tag="vv", name="pos_s")
            nc.sync.dma_start(pos_s[:], pos2_d.ap())
            for b in range(B):
                x_tm = sb.tile([P, ST, D], F32, tag="kb", name="x_tm")
                for c in range(ST):
                    nc.gpsimd.indirect_dma_start(
                        out=x_tm[:, c, :], out_offset=None,
                        in_=emb_d.ap(),
                        in_offset=bass.IndirectOffsetOnAxis(
                            ap=tok_s[:, b * ST + c:b * ST + c + 1], axis=0),
                    )
                nc.vector.tensor_add(x_tm[:], x_tm[:], pos_s[:])
                for c in range(ST):
                    for k in range(DT):
                        tp = pa.tile([P, P], F32, tag="pa", name="tp")
                        nc.tensor.transpose(tp[:], x_tm[:, c, k * P:(k + 1) * P], ident[:])
                        nc.vector.tensor_copy(
                            x_fm[:, k, b * S + c * P:b * S + (c + 1) * P], tp[:])

        # ================= layers =================
        for l in range(L):
            # ---------- attention ----------
            bq_s = sb.tile([P, DT], F32, tag="bq", name="bq_s")
            bk_s = sb.tile([P, DT], F32, tag="bk", name="bk_s")
            bo_s = sb.tile([P, DT], F32, tag="bo", name="bo_s")
            nc.sync.dma_start(bq_s[:], bq_d.ap()[l])
            nc.sync.dma_start(bk_s[:], bk_d.ap()[l])
            nc.sync.dma_start(bo_s[:], bo_d.ap()[l])
            bvrow = sb.tile([1, D], F32, tag="bvrow", name="bvrow")
            nc.sync.dma_start(bvrow[:1, :], bv_d.ap()[l][None, :])
            bv_bc = sb.tile([P, D], F32, tag="bvbc", name="bv_bc")
            nc.gpsimd.partition_broadcast(bv_bc[:], bvrow[:1, :])

            with nc.named_scope(f"attn_{l}"):
                for b in range(B):
                    k_b = sb.tile([P, DT, S], F32R, tag="kb", name="k_b")
                    v_b = sb.tile([P, ST, DT, P], F32R, tag="vv", name="v_b")
                    for ci in range(2):  # 512-token chunks within batch
                        t0 = b * S + ci * CH
                        bc = norm_bc(t0, CH)
                        hn_c = sb.tile([P, DT, CH], F32, tag="normo", name="hn_c")
                        for k in range(DT):
                            nc.vector.tensor_tensor(hn_c[:, k], x_fm[:, k, t0:t0 + CH],
                                                    bc[:], ALU.mult)
                        # q/k projections (fp32) -> f32r
                        q_c = sb.tile([P, DT, CH], F32R, tag="qc", name="q_c")
                        for m in range(DT):
                            wts = sb.tile([P, DT, P], F32, tag="wqk", bufs=2, name="wts")
                            nc.sync.dma_start(wts[:], wq_d.ap()[l, m])
                            pt = pa.tile([P, CH], F32, tag="pa", name="pq")
                            for k in range(DT):
                                nc.tensor.matmul(pt[:], wts[:, k], hn_c[:, k],
                                                 start=(k == 0), stop=(k == DT - 1))
                            nc.vector.tensor_scalar(q_c[:, m], pt[:], bq_s[:, m:m + 1],
                                                    None, ALU.add)
                            wts2 = sb.tile([P, DT, P], F32, tag="wqk", bufs=2, name="wts2")
                            nc.sync.dma_start(wts2[:], wk_d.ap()[l, m])
                            pt2 = pa.tile([P, CH], F32, tag="pa", name="pk")
                            for k in range(DT):
                                nc.tensor.matmul(pt2[:], wts2[:, k], hn_c[:, k],
                                                 start=(k == 0), stop=(k == DT - 1))
                            nc.vector.tensor_scalar(k_b[:, m, ci * CH:(ci + 1) * CH], pt2[:],
                                                    bk_s[:, m:m + 1], None, ALU.add)
                        # v projection (fp32, token-major into head-pair tiles)
                        for j in range(4):  # token tiles in this chunk
                            tt = ci * 4 + j
                            for m in range(DT):  # wv col tiles of 128 = head pair
                                wts3 = sb.tile([P, DT, P], F32, tag="wqk", bufs=2, name="wts3")
                                nc.sync.dma_start(wts3[:], wv_d.ap()[l, m])
                                pt3 = pa.tile([P, P], F32, tag="pa", name="pv")
                                for k in range(DT):
                                    nc.tensor.matmul(pt3[:], hn_c[:, k, j * P:(j + 1) * P],
                                                     wts3[:, k],
                                                     start=(k == 0), stop=(k == DT - 1))
                                nc.vector.tensor_tensor(v_b[:, tt, m, :], pt3[:],
                                                        bv_bc[:, m * P:(m + 1) * P], ALU.add)
                        # attention core (f32r)
                        nkt = 4 * ci + 4
                        att_c = sb.tile([P, DT, CH], F32, tag="attoc", name="att_c")
                        for h in range(H):
                            par, mt = h % 2, h // 2
                            r0 = par * HD
                            pv = ob.tile([P, CH], F32, tag="ob", name="pvp")
                            den = ob.tile([P, CH], F32, tag="ob", name="denp")
                            for kt in range(nkt):
                                ps = pa.tile([P, CH], F32, tag="pa", name="psc")
                                nc.tensor.matmul(
                                    ps[:],
                                    k_b[r0:r0 + HD, mt, kt * P:(kt + 1) * P],
                                    q_c[r0:r0 + HD, mt, :],
                                    start=True, stop=True)
                                pr = sb.tile([P, CH], F32R, tag="probs", bufs=2, name="pr")
                                off = (kt - 4 * ci) * P
                                if off >= 0:  # diagonal block
                                    nc.vector.tensor_tensor(ps[:, off:off + P],
                                                            ps[:, off:off + P], tri[:], ALU.add)
                                    if off > 0:
                                        nc.scalar.mul(pr[:, 0:off], ps[:, 0:off], 0.0)
                                    nc.scalar.activation(pr[:, off:], ps[:, off:], AF.Exp)
                                else:
                                    nc.scalar.activation(pr[:], ps[:], AF.Exp)
                                lhs = v_b[:, kt, mt, :] if par else v_b[:, kt, mt, 0:HD]
                                nc.tensor.matmul(pv[:lhs.shape[-1], :], lhs, pr[:],
                                                 start=(kt == 0), stop=(kt == nkt - 1))
                                nc.tensor.matmul(den[:], ones_r[:], pr[:],
                                                 start=(kt == 0), stop=(kt == nkt - 1))
                            rc = sb.tile([P, CH], F32, tag="rc", bufs=2, name="rc")
                            nc.vector.reciprocal(rc[r0:r0 + HD, :], den[r0:r0 + HD, :])
                            nc.vector.tensor_tensor(att_c[r0:r0 + HD, mt, :],
                                                    pv[r0:r0 + HD, :], rc[r0:r0 + HD, :],
                                                    ALU.mult)
                        # WO (fp32) + residual
                        for m in range(DT):
                            wts4 = sb.tile([P, DT, P], F32, tag="wqk", bufs=2, name="wts4")
                            nc.sync.dma_start(wts4[:], wo_d.ap()[l, m])
                            pt4 = pa.tile([P, CH], F32, tag="pa", name="pwo")
                            for k in range(DT):
                                nc.tensor.matmul(pt4[:], wts4[:, k], att_c[:, k],
                                                 start=(k == 0), stop=(k == DT - 1))
                            st = sb.tile([P, CH], F32, tag="st", bufs=2, name="st")
                            nc.vector.tensor_scalar(st[:], pt4[:], bo_s[:, m:m + 1],
                                                    None, ALU.add)
                            nc.vector.tensor_add(x_fm[:, m, t0:t0 + CH],
                                                 x_fm[:, m, t0:t0 + CH], st[:])

            # ---------- gate + FFN + AllReduce ----------
            gws = sb.tile([P, DT, E], F32, tag="gws", name="gws")
            nc.sync.dma_start(gws[:], gw_d.ap()[l])
            gbrow = sb.tile([1, E], F32, tag="gbrow", name="gbrow")
            nc.sync.dma_start(gbrow[:1, :], gb_d.ap()[l][None, :])
            gb_bc = sb.tile([P, E], F32, tag="gbbc", name="gb_bc")
            nc.gpsimd.partition_broadcast(gb_bc[:], gbrow[:1, :])
            ohrow = sb.tile([1, E], F32, tag="ohrow", name="ohrow")
            nc.sync.dma_start(ohrow[:1, :], oh_d.ap())
            oh_bc = sb.tile([P, E], F32, tag="ohbc", name="oh_bc")
            nc.gpsimd.partition_broadcast(oh_bc[:], ohrow[:1, :])
            b1_s = sb.tile([P, F // P], F32, tag="b1s", name="b1_s")
            nc.sync.dma_start(b1_s[:], b1_d.ap()[l])
            b2_s = sb.tile([P, DT], F32, tag="b2s", name="b2_s")
            nc.sync.dma_start(b2_s[:], b2_d.ap()[l])
            cc_in = dram.tile([D, T], F32, name=f"ccin{l}")
            cc_out = dram.tile([D, T], F32, name=f"ccout{l}", addr_space="Shared")

            with nc.named_scope(f"ffn_{l}"):
                for ci in range(NCH):
                    t0 = ci * CH
                    bc = norm_bc(t0, CH)
                    # gate (fp32) + routing weight for this core's expert
                    hf_c = sb.tile([P, DT, CH], F32, tag="attoc", name="hf_c")
                    hr_c = sb.tile([P, DT, CH], F32R, tag="normo", name="hr_c")
                    for k in range(DT):
                        nc.vector.tensor_tensor(hf_c[:, k], x_fm[:, k, t0:t0 + CH],
                                                bc[:], ALU.mult)
                        nc.vector.tensor_tensor(hr_c[:, k], x_fm[:, k, t0:t0 + CH],
                                                bc[:], ALU.mult)
                    lg = sb.tile([P, 4, E], F32, tag="lg", name="lg")
                    for j in range(4):
                        pg = pa.tile([P, E], F32, tag="pa", name="pg")
                        for k in range(DT):
                            nc.tensor.matmul(pg[:], hf_c[:, k, j * P:(j + 1) * P], gws[:, k],
                                             start=(k == 0), stop=(k == DT - 1))
                        nc.vector.tensor_tensor(lg[:, j], pg[:], gb_bc[:], ALU.add)
                    r1 = sb.tile([P, 4, 1], F32, tag="r1", name="r1")
                    nc.vector.tensor_reduce(r1[:], lg[:], axis=AX.X, op=ALU.max)
                    eq1 = sb.tile([P, 4, E], F32, tag="eq1", name="eq1")
                    nc.vector.tensor_tensor(eq1[:], lg[:], r1[:].to_broadcast([P, 4, E]),
                                            ALU.is_equal)
                    msk = sb.tile([P, 4, E], F32, tag="msk", name="msk")
                    nc.vector.tensor_scalar(msk[:], eq1[:], 1e30, None, ALU.mult)
                    nc.vector.tensor_sub(msk[:], lg[:], msk[:])
                    r2 = sb.tile([P, 4, 1], F32, tag="r2", name="r2")
                    nc.vector.tensor_reduce(r2[:], msk[:], axis=AX.X, op=ALU.max)
                    sig = sb.tile([P, 4, 1], F32, tag="sig", name="sig")
                    nc.vector.tensor_sub(sig[:], r1[:], r2[:])
                    nc.scalar.activation(sig[:], sig[:], AF.Sigmoid)
                    eq2 = sb.tile([P, 4, E], F32, tag="eq2", name="eq2")
                    nc.vector.tensor_tensor(eq2[:], lg[:], r2[:].to_broadcast([P, 4, E]),
                                            ALU.is_equal)
                    nc.vector.tensor_tensor(eq1[:], eq1[:], sig[:].to_broadcast([P, 4, E]),
                                            ALU.mult)
                    nc.vector.tensor_scalar(sig[:], sig[:], -1.0, 1.0, ALU.mult, ALU.add)
                    nc.vector.tensor_tensor(eq2[:], eq2[:], sig[:].to_broadcast([P, 4, E]),
                                            ALU.mult)
                    nc.vector.tensor_add(eq1[:], eq1[:], eq2[:])
                    nc.vector.tensor_tensor(eq1[:], eq1[:],
                                            oh_bc[:, None, :].to_broadcast([P, 4, E]),
                                            ALU.mult)
                    cw_tm = sb.tile([P, 4], F32, tag="cwtm", name="cw_tm")
                    nc.vector.tensor_reduce(cw_tm[:], eq1[:], axis=AX.X, op=ALU.add)
                    cwd = dram.tile([CH], F32, name=f"cwd{l}_{ci}")
                    nc.sync.dma_start(cwd[:].rearrange("(c p) -> p c", p=P), cw_tm[:])
                    cwrow = sb.tile([1, CH], F32, tag="cwrow", bufs=1, name="cwrow")
                    nc.sync.dma_start(cwrow[:1, :], cwd[:][None, :])
                    cw_bc = sb.tile([P, CH], F32, tag="cwbc", bufs=2, name="cw_bc")
                    nc.gpsimd.partition_broadcast(cw_bc[:], cwrow[:1, :])

                    # FFN: PSUM-resident accumulation over all 12 bands
                    outp = [ob.tile([P, CH], F32, tag="ob", name=f"po{d}") for d in range(DT)]
                    for n in range(NB):
                        w1b = sb.tile([P, DT, FB], F32R, tag="w1b", bufs=2, name="w1b")
                        nc.sync.dma_start(w1b[:], w1_d.ap()[l, n])
                        w2b = sb.tile([P, FT, D], F32R, tag="w2b", bufs=2, name="w2b")
                        nc.sync.dma_start(w2b[:], w2_d.ap()[l, n])
                        h1 = sb.tile([P, FT, CH], F32R, tag="h1", bufs=2, name="h1")
                        for f in range(FT):
                            ph = pa.tile([P, CH], F32, tag="pa", name="ph1")
                            for k in range(DT):
                                nc.tensor.matmul(ph[:], w1b[:, k, f * P:(f + 1) * P], hr_c[:, k],
                                                 start=(k == 0), stop=(k == DT - 1))
                            nc.scalar.activation(h1[:, f], ph[:], AF.Gelu,
                                                 bias=b1_s[:, n * FT + f:n * FT + f + 1])
                        for d in range(DT):
                            for f in range(FT):
                                nc.tensor.matmul(outp[d][:], w2b[:, f, d * P:(d + 1) * P],
                                                 h1[:, f],
                                                 start=(n == 0 and f == 0),
                                                 stop=(n == NB - 1 and f == FT - 1))
                    for d in range(DT):
                        st = sb.tile([P, CH], F32, tag="st", bufs=2, name="stf")
                        nc.vector.tensor_scalar(st[:], outp[d][:], b2_s[:, d:d + 1],
                                                None, ALU.add)
                        nc.vector.tensor_tensor(st[:], st[:], cw_bc[:], ALU.mult)
                        nc.sync.dma_start(fm(cc_in[:])[:, d, t0:t0 + CH], st[:])

            with nc.named_scope(f"ar_{l}"):
                nc.gpsimd.collective_compute(
                    "AllReduce", ALU.add,
                    replica_groups=[list(range(CORES))],
                    ins=[cc_in[:]], outs=[cc_out[:]],
                )
                for d in range(DT):
                    for ci in range(NCH):
                        st = sb.tile([P, CH], F32, tag="st", bufs=2, name="sta")
                        nc.sync.dma_start(st[:], fm(cc_out[:])[:, d, ci * CH:(ci + 1) * CH])
                        nc.vector.tensor_add(x_fm[:, d, ci * CH:(ci + 1) * CH],
                                             x_fm[:, d, ci * CH:(ci + 1) * CH], st[:])

        # ================= final norm + LM head =================
        with nc.named_scope("head"):
            xf = [sb.tile([P, DT, S], F32R, tag="kb", name="xf_a"),
                  sb.tile([P, DT, S], F32R, tag="vv", name="xf_b")]
            for ci in range(NCH):
                t0 = ci * CH
                bc = norm_bc(t0, CH)
                half, o2 = divmod(t0, S)
                for k in range(DT):
                    nc.vector.tensor_tensor(xf[half][:, k, o2:o2 + CH],
                                            x_fm[:, k, t0:t0 + CH], bc[:], ALU.mult)
            for vc in range(NVCH):
                ehs = sb.tile([P, DT, VCH], F32R, tag=("qc" if vc % 2 == 0 else "attoc"),
                              bufs=1, name="ehs")
                nc.sync.dma_start(ehs[:], eh_d.ap()[vc])
                for c in range(TT):
                    half, ct = divmod(c, ST)
                    pt = pa.tile([P, VCH], F32, tag="pa", name="phd")
                    for k in range(DT):
                        nc.tensor.matmul(pt[:], xf[half][:, k, ct * P:(ct + 1) * P],
                                         ehs[:, k],
                                         start=(k == 0), stop=(k == DT - 1))
                    st = sb.tile([P, VCH], F32, tag="st", bufs=2, name="sth")
                    nc.vector.tensor_copy(st[:], pt[:])
                    nc.sync.dma_start(
                        out_d.ap().rearrange("(c p) v -> p c v", p=P)[:, c, vc * VCH:(vc + 1) * VCH],
                        st[:])

    nc.compile()
    return nc


def _prep_inputs(inputs):
    gi = {k: np.asarray(v) for k, v in inputs.items()}
    ln1 = gi["ln1"].astype(np.float32)
    ln2 = gi["ln2"].astype(np.float32)
    lnf = gi["ln_f"].astype(np.float32)
    emb = np.ascontiguousarray(gi["emb"], np.float32)
    rs = np.float32(np.sqrt(HD))

    def packw(w):
        # [L, Din, Dout] -> [L, m, p, k, j]: out[l,m,p,k,j] = w[l, k*128+p, m*128+j]
        r = w.reshape(L, DT, P, DT, P)            # [l, k, p, m, j]
        return np.ascontiguousarray(r.transpose(0, 3, 2, 1, 4), np.float32)

    def packb(b):
        # [L, D] -> [L, p, m]
        return np.ascontiguousarray(b.reshape(L, DT, P).transpose(0, 2, 1), np.float32)

    pos = np.asarray(gi["pos_emb"], np.float32)
    common = {
        "tok32": np.ascontiguousarray(gi["tokens"].reshape(T).astype(np.int32)),
        "emb": emb,
        "pos": pos,
        "posp": np.ascontiguousarray(pos.reshape(ST, P, D).transpose(1, 0, 2), np.float32),
        "wq": packw(ln1[:, :, None] * gi["wq"] / rs),
        "wk": packw(ln1[:, :, None] * gi["wk"]),
        "wv": packw(ln1[:, :, None] * gi["wv"]),
        "wo": packw(np.asarray(gi["wo"], np.float32)),
        "bq": packb(gi["bq"] / rs),
        "bk": packb(gi["bk"]),
        "bv": np.ascontiguousarray(gi["bv"], np.float32),
        "bo": packb(gi["bo"]),
        "gw": np.ascontiguousarray(
            (ln2[:, :, None] * gi["gate_w"]).reshape(L, DT, P, E).transpose(0, 2, 1, 3), np.float32),
        "gb": np.ascontiguousarray(gi["gate_b"], np.float32),
    }
    in_maps = []
    for c in range(CORES):
        onehot = np.zeros((1, E), np.float32)
        onehot[0, c] = 1.0
        m = dict(common)
        w1 = (ln2[:, :, None] * gi["w1"][:, c]).reshape(L, DT, P, NB, FB)
        m["w1e"] = np.ascontiguousarray(w1.transpose(0, 3, 2, 1, 4), np.float32)
        m["b1e"] = np.ascontiguousarray(
            gi["b1"][:, c].reshape(L, F // P, P).transpose(0, 2, 1), np.float32)
        w2 = np.asarray(gi["w2"][:, c], np.float32).reshape(L, NB, FT, P, D)
        m["w2e"] = np.ascontiguousarray(w2.transpose(0, 1, 3, 2, 4), np.float32)
        m["b2e"] = np.ascontiguousarray(
            gi["b2"][:, c].reshape(L, DT, P).transpose(0, 2, 1), np.float32)
        m["onehot"] = onehot
        eT = (emb[c * VS:(c + 1) * VS, :] * lnf[None, :]).T  # [D, VS]
        eT = eT.reshape(DT, P, NVCH, VCH)
        m["embT"] = np.ascontiguousarray(eT.transpose(2, 1, 0, 3), np.float32)
        in_maps.append(m)
    return in_maps


def kernel(**inputs):
    global _compiled
    if _compiled is None:
        _compiled = build()
    in_maps = _prep_inputs(inputs)
    res = run_bass_kernel_spmd(_compiled, in_maps, core_ids=list(range(CORES)))
    shards = [res.results[c]["out"].reshape(B, S, VS) for c in range(CORES)]
    return np.concatenate(shards, axis=-1)


if __name__ == "__main__":
    import reference as R
    inputs = R.setup_inputs()
    out = kernel(**{k: np.asarray(v) for k, v in inputs.items()})
    print("kernel out", out.shape, out.dtype)


# revision 8
# speedup vs baseline: 1.0734x; 1.0503x over previous
"""MoE transformer (B=2,S=1024,D=768,H=12,F=3072,E=8,K=2,V=32000,L=2) on 8 TRN2 cores.

Sharding: expert-parallel (core c owns expert c) + vocab-parallel LM head
(core c computes logits for vocab [c*4000,(c+1)*4000)). Attention/embedding
replicated. One AllReduce of the cw-scaled expert output per layer.

Precision: fp32 for everything feeding the router logits (QKV/WO/gate);
float32r (TF32-class, full PE speed) for attention scores/PV, both FFNs and
the LM head. This keeps top-2 expert selection identical to the fp32
reference (min routing margin 2.6e-5 >> f32r-path noise) while running the
FLOP-heavy parts at ~4.6x the fp32 matmul rate.

Layout: residual stream feature-major [768(part), 2048(tok)], token
t = c*128 + p. Attention computed transposed (S_T[k,q]); softmax denominator
comes from an all-ones-lhsT matmul that broadcasts the column sums to all
PSUM partitions; causal blocks above the diagonal are skipped. Heads are
paired in 128-row tiles: even head values in v cols 0:64, odd in 64:128, so
PV output lands at the partition base its head needs.
"""
import sys

sys.path.insert(0, "/opt/trn_rl_repo")

import numpy as np

import concourse.bass as bass
import concourse.mybir as mybir
import concourse.tile as tile
from concourse import bacc
from concourse.bass_utils import run_bass_kernel_spmd
from concourse.masks import make_identity, make_upper_triangular

B, S, D, H, F, E, TOPK, VOC, L = 2, 1024, 768, 12, 3072, 8, 2, 32000, 2
T = B * S
P = 128
HD = D // H               # 64
DT = D // P               # 6
TT = T // P               # 16
ST = S // P               # 8
CORES = 8
VS = VOC // CORES         # 4000
VCH = 500
NVCH = VS // VCH          # 8
CH = 512                  # token chunk
NCH = T // CH             # 4 global chunks
FB = 256                  # ff band
NB = F // FB              # 12 bands
FT = FB // P              # 2
EPS = 1e-6

F32 = mybir.dt.float32
F32R = mybir.dt.float32r
I32 = mybir.dt.int32
AF = mybir.ActivationFunctionType
ALU = mybir.AluOpType
AX = mybir.AxisListType

_compiled = None


def fm(ap):
    """[(o p), rest...] -> [p, o, rest...] with p=128 (feature-major load)."""
    letters = "abcdefg"[: len(ap.shape) - 1]
    src = "(o p) " + " ".join(letters)
    dst = "p o " + " ".join(letters)
    return ap.rearrange(f"{src} -> {dst}", p=P)


def build():
    nc = bacc.Bacc(None, target_bir_lowering=False, debug=False)

    tok_d = nc.dram_tensor("tok32", [T], I32, kind="ExternalInput")
    emb_d = nc.dram_tensor("emb", [VOC, D], F32, kind="ExternalInput")
    pos_d = nc.dram_tensor("pos", [S, D], F32, kind="ExternalInput")
    # weights pre-packed on host into SBUF layouts (contiguous DMA)
    wq_d = nc.dram_tensor("wq", [L, DT, P, DT, P], F32, kind="ExternalInput")
    wk_d = nc.dram_tensor("wk", [L, DT, P, DT, P], F32, kind="ExternalInput")
    wv_d = nc.dram_tensor("wv", [L, DT, P, DT, P], F32, kind="ExternalInput")
    wo_d = nc.dram_tensor("wo", [L, DT, P, DT, P], F32, kind="ExternalInput")
    bq_d = nc.dram_tensor("bq", [L, P, DT], F32, kind="ExternalInput")
    bk_d = nc.dram_tensor("bk", [L, P, DT], F32, kind="ExternalInput")
    bv_d = nc.dram_tensor("bv", [L, P, D], F32, kind="ExternalInput")   # pre-bcast
    bo_d = nc.dram_tensor("bo", [L, P, DT], F32, kind="ExternalInput")
    gw_d = nc.dram_tensor("gw", [L, P, DT, E], F32, kind="ExternalInput")
    gb_d = nc.dram_tensor("gb", [L, P, E], F32, kind="ExternalInput")   # pre-bcast
    w1_d = nc.dram_tensor("w1e", [L, NB, P, DT, FB], F32R, kind="ExternalInput")
    b1_d = nc.dram_tensor("b1e", [L, P, F // P], F32, kind="ExternalInput")
    w2_d = nc.dram_tensor("w2e", [L, NB, P, FT, D], F32R, kind="ExternalInput")
    b2_d = nc.dram_tensor("b2e", [L, P, DT], F32, kind="ExternalInput")
    oh_d = nc.dram_tensor("onehot", [P, E], F32, kind="ExternalInput")  # pre-bcast
    eh_d = nc.dram_tensor("embT", [NVCH, P, DT, VCH], F32R, kind="ExternalInput")
    pos2_d = nc.dram_tensor("posp", [P, ST, D], F32, kind="ExternalInput")
    out_d = nc.dram_tensor("out", [T, VS], F32, kind="ExternalOutput")

    with tile.TileContext(nc) as tc, \
         tc.tile_pool(name="sb", bufs=1) as sb, \
         tc.tile_pool(name="pa", bufs=2, space="PSUM") as pa, \
         tc.tile_pool(name="ob", bufs=6, space="PSUM") as ob, \
         tc.tile_pool(name="dram", bufs=1, space="DRAM") as dram:

        # ---- constants ----
        ident = sb.tile([P, P], F32)
        make_identity(nc, ident)
        tri = sb.tile([P, P], F32)   # additive causal mask in [k,q]: 0 if q>=k else -1e9
        make_upper_triangular(nc, tri, val=1.0, diag=True)
        nc.vector.tensor_scalar(tri[:], tri[:], 1.0, 1e9, ALU.subtract, ALU.mult)
        onesf = sb.tile([P, P], F32)
        nc.gpsimd.memset(onesf[:], 1.0)
        ones_r = sb.tile([P, P], F32R)
        nc.vector.tensor_copy(ones_r[:], onesf[:])
        eps_t = sb.tile([P, 1], F32)
        nc.gpsimd.memset(eps_t[:], EPS)

        # ---- big persistent / overlaid tiles ----
        x_fm = sb.tile([P, DT, T], F32, tag="x")

        def norm_bc(t0, width, tag="nbc"):
            """rstd broadcast tile for x tokens [t0,t0+width)."""
            pr = pa.tile([P, width], F32, tag="pa", name="prn")
            for k in range(DT):
                sq = sb.tile([P, width], F32R, tag="probs", bufs=2, name="sq")
                nc.scalar.activation(sq[:], x_fm[:, k, t0:t0 + width], AF.Square)
                nc.tensor.matmul(pr[:1, :], ones_r[:, :1], sq[:],
                                 start=(k == 0), stop=(k == DT - 1))
            srow = sb.tile([1, width], F32, tag="srow", bufs=1, name="srow")
            nc.scalar.activation(srow[:1, :], pr[:1, :], AF.Sqrt, bias=eps_t[:1, :], scale=1.0 / D)
            rrow = sb.tile([1, width], F32, tag="rrow", bufs=1, name="rrow")
            nc.vector.reciprocal(rrow[:1, :], srow[:1, :])
            bc = sb.tile([P, width], F32, tag=tag, bufs=2, name="bc")
            nc.gpsimd.partition_broadcast(bc[:], rrow[:1, :])
            return bc

        # ================= embedding =================
        with nc.named_scope("embed"):
            tok_s = sb.tile([P, TT], I32, tag="toks")
            nc.sync.dma_start(tok_s[:], tok_d.ap().rearrange("(c p) -> p c", p=P))
            pos_s = sb.tile([P, ST, D], F32, tag="vv", name="pos_s")
            nc.sync.dma_start(pos_s[:], pos2_d.ap())
            for b in range(B):
                x_tm = sb.tile([P, ST, D], F32, tag="kb", name="x_tm")
                for c in range(ST):
                    nc.gpsimd.indirect_dma_start(
                        out=x_tm[:, c, :], out_offset=None,
                        in_=emb_d.ap(),
                        in_offset=bass.IndirectOffsetOnAxis(
                            ap=tok_s[:, b * ST + c:b * ST + c + 1], axis=0),
                    )
                nc.vector.tensor_add(x_tm[:], x_tm[:], pos_s[:])
                for c in range(ST):
                    for k in range(DT):
                        tp = pa.tile([P, P], F32, tag="pa", name="tp")
                        nc.tensor.transpose(tp[:], x_tm[:, c, k * P:(k + 1) * P], ident[:])
                        nc.vector.tensor_copy(
                            x_fm[:, k, b * S + c * P:b * S + (c + 1) * P], tp[:])

        # ================= layers =================
        for l in range(L):
            # ---------- attention ----------
            bq_s = sb.tile([P, DT], F32, tag="bq", name="bq_s")
            bk_s = sb.tile([P, DT], F32, tag="bk", name="bk_s")
            bo_s = sb.tile([P, DT], F32, tag="bo", name="bo_s")
            nc.sync.dma_start(bq_s[:], bq_d.ap()[l])
            nc.sync.dma_start(bk_s[:], bk_d.ap()[l])
            nc.sync.dma_start(bo_s[:], bo_d.ap()[l])
            bv_bc = sb.tile([P, D], F32, tag="bvbc", name="bv_bc")
            nc.sync.dma_start(bv_bc[:], bv_d.ap()[l])

            with nc.named_scope(f"attn_{l}"):
                for b in range(B):
                    k_b = sb.tile([P, DT, S], F32R, tag="kb", name="k_b")
                    v_b = sb.tile([P, ST, DT, P], F32R, tag="vv", name="v_b")
                    for ci in range(2):  # 512-token chunks within batch
                        t0 = b * S + ci * CH
                        bc = norm_bc(t0, CH)
                        hn_c = sb.tile([P, DT, CH], F32, tag="normo", bufs=2, name="hn_c")
                        for k in range(DT):
                            nc.vector.tensor_tensor(hn_c[:, k], x_fm[:, k, t0:t0 + CH],
                                                    bc[:], ALU.mult)
                        # q/k projections (fp32) -> f32r
                        q_c = sb.tile([P, DT, CH], F32R, tag="qc", name="q_c")
                        for m in range(DT):
                            wts = sb.tile([P, DT, P], F32, tag="wqk", bufs=2, name="wts")
                            nc.sync.dma_start(wts[:], wq_d.ap()[l, m])
                            pt = pa.tile([P, CH], F32, tag="pa", name="pq")
                            for k in range(DT):
                                nc.tensor.matmul(pt[:], wts[:, k], hn_c[:, k],
                                                 start=(k == 0), stop=(k == DT - 1))
                            nc.vector.tensor_scalar(q_c[:, m], pt[:], bq_s[:, m:m + 1],
                                                    None, ALU.add)
                            wts2 = sb.tile([P, DT, P], F32, tag="wqk", bufs=2, name="wts2")
                            nc.sync.dma_start(wts2[:], wk_d.ap()[l, m])
                            pt2 = pa.tile([P, CH], F32, tag="pa", name="pk")
                            for k in range(DT):
                                nc.tensor.matmul(pt2[:], wts2[:, k], hn_c[:, k],
                                                 start=(k == 0), stop=(k == DT - 1))
                            nc.vector.tensor_scalar(k_b[:, m, ci * CH:(ci + 1) * CH], pt2[:],
                                                    bk_s[:, m:m + 1], None, ALU.add)
                        # v projection (fp32, token-major into head-pair tiles)
                        for j in range(4):  # token tiles in this chunk
                            tt = ci * 4 + j
                            for m in range(DT):  # wv col tiles of 128 = head pair
                                wts3 = sb.tile([P, DT, P], F32, tag="wqk", bufs=2, name="wts3")
                                nc.sync.dma_start(wts3[:], wv_d.ap()[l, m])
                                pt3 = pa.tile([P, P], F32, tag="pa", name="pv")
                                for k in range(DT):
                                    nc.tensor.matmul(pt3[:], hn_c[:, k, j * P:(j + 1) * P],
                                                     wts3[:, k],
                                                     start=(k == 0), stop=(k == DT - 1))
                                nc.vector.tensor_tensor(v_b[:, tt, m, :], pt3[:],
                                                        bv_bc[:, m * P:(m + 1) * P], ALU.add)
                        # attention core (f32r)
                        nkt = 4 * ci + 4
                        att_c = sb.tile([P, DT, CH], F32, tag="attoc", name="att_c")
                        for h in range(H):
                            par, mt = h % 2, h // 2
                            r0 = par * HD
                            pv = ob.tile([P, CH], F32, tag="ob", name="pvp")
                            den = ob.tile([P, CH], F32, tag="ob", name="denp")
                            for kt in range(nkt):
                                ps = pa.tile([P, CH], F32, tag="pa", name="psc")
                                nc.tensor.matmul(
                                    ps[:],
                                    k_b[r0:r0 + HD, mt, kt * P:(kt + 1) * P],
                                    q_c[r0:r0 + HD, mt, :],
                                    start=True, stop=True)
                                pr = sb.tile([P, CH], F32R, tag="probs", bufs=2, name="pr")
                                off = (kt - 4 * ci) * P
                                if off >= 0:  # diagonal block
                                    nc.vector.tensor_tensor(ps[:, off:off + P],
                                                            ps[:, off:off + P], tri[:], ALU.add)
                                    if off > 0:
                                        nc.scalar.mul(pr[:, 0:off], ps[:, 0:off], 0.0)
                                    nc.scalar.activation(pr[:, off:], ps[:, off:], AF.Exp)
                                else:
                                    nc.scalar.activation(pr[:], ps[:], AF.Exp)
                                lhs = v_b[:, kt, mt, :] if par else v_b[:, kt, mt, 0:HD]
                                nc.tensor.matmul(pv[:lhs.shape[-1], :], lhs, pr[:],
                                                 start=(kt == 0), stop=(kt == nkt - 1))
                                nc.tensor.matmul(den[:], ones_r[:], pr[:],
                                                 start=(kt == 0), stop=(kt == nkt - 1))
                            rc = sb.tile([P, CH], F32, tag="rc", bufs=1, name="rc")
                            nc.vector.reciprocal(rc[r0:r0 + HD, :], den[r0:r0 + HD, :])
                            nc.vector.tensor_tensor(att_c[r0:r0 + HD, mt, :],
                                                    pv[r0:r0 + HD, :], rc[r0:r0 + HD, :],
                                                    ALU.mult)
                        # WO (fp32) + residual
                        for m in range(DT):
                            wts4 = sb.tile([P, DT, P], F32, tag="wqk", bufs=2, name="wts4")
                            nc.sync.dma_start(wts4[:], wo_d.ap()[l, m])
                            pt4 = pa.tile([P, CH], F32, tag="pa", name="pwo")
                            for k in range(DT):
                                nc.tensor.matmul(pt4[:], wts4[:, k], att_c[:, k],
                                                 start=(k == 0), stop=(k == DT - 1))
                            st = sb.tile([P, CH], F32, tag="st", bufs=2, name="st")
                            nc.vector.tensor_scalar(st[:], pt4[:], bo_s[:, m:m + 1],
                                                    None, ALU.add)
                            nc.vector.tensor_add(x_fm[:, m, t0:t0 + CH],
                                                 x_fm[:, m, t0:t0 + CH], st[:])

            # ---------- gate + FFN + AllReduce ----------
            gws = sb.tile([P, DT, E], F32, tag="gws", name="gws")
            nc.sync.dma_start(gws[:], gw_d.ap()[l])
            gb_bc = sb.tile([P, E], F32, tag="gbbc", name="gb_bc")
            nc.sync.dma_start(gb_bc[:], gb_d.ap()[l])
            oh_bc = sb.tile([P, E], F32, tag="ohbc", name="oh_bc")
            nc.sync.dma_start(oh_bc[:], oh_d.ap())
            b1_s = sb.tile([P, F // P], F32, tag="b1s", name="b1_s")
            nc.sync.dma_start(b1_s[:], b1_d.ap()[l])
            b2_s = sb.tile([P, DT], F32, tag="b2s", name="b2_s")
            nc.sync.dma_start(b2_s[:], b2_d.ap()[l])
            with nc.named_scope(f"ffn_{l}"):
                for ci in range(NCH):
                    t0 = ci * CH
                    cc_in = dram.tile([D, CH], F32, name=f"ccin{l}_{ci}")
                    cc_out = dram.tile([D, CH], F32, name=f"ccout{l}_{ci}", addr_space="Shared")
                    bc = norm_bc(t0, CH)
                    # gate (fp32) + routing weight for this core's expert
                    hf_c = sb.tile([P, DT, CH], F32, tag="attoc", name="hf_c")
                    hr_c = sb.tile([P, DT, CH], F32R, tag="normo", bufs=2, name="hr_c")
                    for k in range(DT):
                        nc.vector.tensor_tensor(hf_c[:, k], x_fm[:, k, t0:t0 + CH],
                                                bc[:], ALU.mult)
                        nc.vector.tensor_tensor(hr_c[:, k], x_fm[:, k, t0:t0 + CH],
                                                bc[:], ALU.mult)
                    lg = sb.tile([P, 4, E], F32, tag="lg", name="lg")
                    for j in range(4):
                        pg = pa.tile([P, E], F32, tag="pa", name="pg")
                        for k in range(DT):
                            nc.tensor.matmul(pg[:], hf_c[:, k, j * P:(j + 1) * P], gws[:, k],
                                             start=(k == 0), stop=(k == DT - 1))
                        nc.vector.tensor_tensor(lg[:, j], pg[:], gb_bc[:], ALU.add)
                    r1 = sb.tile([P, 4, 1], F32, tag="r1", name="r1")
                    nc.vector.tensor_reduce(r1[:], lg[:], axis=AX.X, op=ALU.max)
                    eq1 = sb.tile([P, 4, E], F32, tag="eq1", name="eq1")
                    nc.vector.tensor_tensor(eq1[:], lg[:], r1[:].to_broadcast([P, 4, E]),
                                            ALU.is_equal)
                    msk = sb.tile([P, 4, E], F32, tag="msk", name="msk")
                    nc.vector.tensor_scalar(msk[:], eq1[:], 1e30, None, ALU.mult)
                    nc.vector.tensor_sub(msk[:], lg[:], msk[:])
                    r2 = sb.tile([P, 4, 1], F32, tag="r2", name="r2")
                    nc.vector.tensor_reduce(r2[:], msk[:], axis=AX.X, op=ALU.max)
                    sig = sb.tile([P, 4, 1], F32, tag="sig", name="sig")
                    nc.vector.tensor_sub(sig[:], r1[:], r2[:])
                    nc.scalar.activation(sig[:], sig[:], AF.Sigmoid)
                    eq2 = sb.tile([P, 4, E], F32, tag="eq2", name="eq2")
                    nc.vector.tensor_tensor(eq2[:], lg[:], r2[:].to_broadcast([P, 4, E]),
                                            ALU.is_equal)
                    nc.vector.tensor_tensor(eq1[:], eq1[:], sig[:].to_broadcast([P, 4, E]),
                                            ALU.mult)
                    nc.vector.tensor_scalar(sig[:], sig[:], -1.0, 1.0, ALU.mult, ALU.add)
                    nc.vector.tensor_tensor(eq2[:], eq2[:], sig[:].to_broadcast([P, 4, E]),
                                            ALU.mult)
                    nc.vector.tensor_add(eq1[:], eq1[:], eq2[:])
                    nc.vector.tensor_tensor(eq1[:], eq1[:],
                                            oh_bc[:, None, :].to_broadcast([P, 4, E]),
                                            ALU.mult)
                    cw_tm = sb.tile([P, 4], F32, tag="cwtm", name="cw_tm")
                    nc.vector.tensor_reduce(cw_tm[:], eq1[:], axis=AX.X, op=ALU.add)
                    cwd = dram.tile([CH], F32, name=f"cwd{l}_{ci}")
                    nc.sync.dma_start(cwd[:].rearrange("(c p) -> p c", p=P), cw_tm[:])
                    cwrow = sb.tile([1, CH], F32, tag="cwrow", bufs=1, name="cwrow")
                    nc.sync.dma_start(cwrow[:1, :], cwd[:][None, :])
                    cw_bc = sb.tile([P, CH], F32, tag="cwbc", bufs=1, name="cw_bc")
                    nc.gpsimd.partition_broadcast(cw_bc[:], cwrow[:1, :])

                    # FFN: PSUM-resident accumulation over all 12 bands
                    outp = [ob.tile([P, CH], F32, tag="ob", name=f"po{d}") for d in range(DT)]
                    for n in range(NB):
                        w1b = sb.tile([P, DT, FB], F32R, tag="w1b", bufs=2, name="w1b")
                        nc.sync.dma_start(w1b[:], w1_d.ap()[l, n])
                        w2b = sb.tile([P, FT, D], F32R, tag="w2b", bufs=2, name="w2b")
                        nc.sync.dma_start(w2b[:], w2_d.ap()[l, n])
                        h1 = sb.tile([P, FT, CH], F32R, tag="h1", bufs=1, name="h1")
                        for f in range(FT):
                            ph = pa.tile([P, CH], F32, tag="pa", name="ph1")
                            for k in range(DT):
                                nc.tensor.matmul(ph[:], w1b[:, k, f * P:(f + 1) * P], hr_c[:, k],
                                                 start=(k == 0), stop=(k == DT - 1))
                            nc.scalar.activation(h1[:, f], ph[:], AF.Gelu,
                                                 bias=b1_s[:, n * FT + f:n * FT + f + 1])
                        for d in range(DT):
                            for f in range(FT):
                                nc.tensor.matmul(outp[d][:], w2b[:, f, d * P:(d + 1) * P],
                                                 h1[:, f],
                                                 start=(n == 0 and f == 0),
                                                 stop=(n == NB - 1 and f == FT - 1))
                    for d in range(DT):
                        st = sb.tile([P, CH], F32, tag="st", bufs=2, name="stf")
                        nc.vector.tensor_scalar(st[:], outp[d][:], b2_s[:, d:d + 1],
                                                None, ALU.add)
                        nc.vector.tensor_tensor(st[:], st[:], cw_bc[:], ALU.mult)
                        nc.sync.dma_start(fm(cc_in[:])[:, d, :], st[:])

                    nc.gpsimd.collective_compute(
                        "AllReduce", ALU.add,
                        replica_groups=[list(range(CORES))],
                        ins=[cc_in[:]], outs=[cc_out[:]],
                    )
                    for d in range(DT):
                        st = sb.tile([P, CH], F32, tag="st", bufs=2, name="sta")
                        nc.sync.dma_start(st[:], fm(cc_out[:])[:, d, :])
                        nc.vector.tensor_add(x_fm[:, d, t0:t0 + CH],
                                             x_fm[:, d, t0:t0 + CH], st[:])

        # ================= final norm + LM head =================
        with nc.named_scope("head"):
            xf = [sb.tile([P, DT, S], F32R, tag="kb", name="xf_a"),
                  sb.tile([P, DT, S], F32R, tag="vv", name="xf_b")]
            for ci in range(NCH):
                t0 = ci * CH
                bc = norm_bc(t0, CH)
                half, o2 = divmod(t0, S)
                for k in range(DT):
                    nc.vector.tensor_tensor(xf[half][:, k, o2:o2 + CH],
                                            x_fm[:, k, t0:t0 + CH], bc[:], ALU.mult)
            for vc in range(NVCH):
                ehs = sb.tile([P, DT, VCH], F32R, tag=("qc" if vc % 2 == 0 else "attoc"),
                              bufs=1, name="ehs")
                nc.sync.dma_start(ehs[:], eh_d.ap()[vc])
                for c in range(TT):
                    half, ct = divmod(c, ST)
                    pt = ob.tile([P, VCH], F32, tag="ob", name="phd")
                    for k in range(DT):
                        nc.tensor.matmul(pt[:], xf[half][:, k, ct * P:(ct + 1) * P],
                                         ehs[:, k],
                                         start=(k == 0), stop=(k == DT - 1))
                    st = sb.tile([P, VCH], F32, tag="st", bufs=2, name="sth")
                    nc.vector.tensor_copy(st[:], pt[:])
                    nc.sync.dma_start(
                        out_d.ap().rearrange("(c p) v -> p c v", p=P)[:, c, vc * VCH:(vc + 1) * VCH],
                        st[:])

    nc.compile()
    return nc


def _prep_inputs(inputs):
    gi = {k: np.asarray(v) for k, v in inputs.items()}
    ln1 = gi["ln1"].astype(np.float32)
    ln2 = gi["ln2"].astype(np.float32)
    lnf = gi["ln_f"].astype(np.float32)
    emb = np.ascontiguousarray(gi["emb"], np.float32)
    rs = np.float32(np.sqrt(HD))

    def packw(w):
        # [L, Din, Dout] -> [L, m, p, k, j]: out[l,m,p,k,j] = w[l, k*128+p, m*128+j]
        r = w.reshape(L, DT, P, DT, P)            # [l, k, p, m, j]
        return np.ascontiguousarray(r.transpose(0, 3, 2, 1, 4), np.float32)

    def packb(b):
        # [L, D] -> [L, p, m]
        return np.ascontiguousarray(b.reshape(L, DT, P).transpose(0, 2, 1), np.float32)

    pos = np.asarray(gi["pos_emb"], np.float32)
    common = {
        "tok32": np.ascontiguousarray(gi["tokens"].reshape(T).astype(np.int32)),
        "emb": emb,
        "pos": pos,
        "posp": np.ascontiguousarray(pos.reshape(ST, P, D).transpose(1, 0, 2), np.float32),
        "wq": packw(ln1[:, :, None] * gi["wq"] / rs),
        "wk": packw(ln1[:, :, None] * gi["wk"]),
        "wv": packw(ln1[:, :, None] * gi["wv"]),
        "wo": packw(np.asarray(gi["wo"], np.float32)),
        "bq": packb(gi["bq"] / rs),
        "bk": packb(gi["bk"]),
        "bv": np.ascontiguousarray(np.broadcast_to(gi["bv"][:, None, :], (L, P, D)), np.float32),
        "bo": packb(gi["bo"]),
        "gw": np.ascontiguousarray(
            (ln2[:, :, None] * gi["gate_w"]).reshape(L, DT, P, E).transpose(0, 2, 1, 3), np.float32),
        "gb": np.ascontiguousarray(np.broadcast_to(gi["gate_b"][:, None, :], (L, P, E)), np.float32),
    }
    in_maps = []
    for c in range(CORES):
        onehot = np.zeros((P, E), np.float32)
        onehot[:, c] = 1.0
        m = dict(common)
        w1 = (ln2[:, :, None] * gi["w1"][:, c]).reshape(L, DT, P, NB, FB)
        m["w1e"] = np.ascontiguousarray(w1.transpose(0, 3, 2, 1, 4), np.float32)
        m["b1e"] = np.ascontiguousarray(
            gi["b1"][:, c].reshape(L, F // P, P).transpose(0, 2, 1), np.float32)
        w2 = np.asarray(gi["w2"][:, c], np.float32).reshape(L, NB, FT, P, D)
        m["w2e"] = np.ascontiguousarray(w2.transpose(0, 1, 3, 2, 4), np.float32)
        m["b2e"] = np.ascontiguousarray(
            gi["b2"][:, c].reshape(L, DT, P).transpose(0, 2, 1), np.float32)
        m["onehot"] = onehot
        eT = (emb[c * VS:(c + 1) * VS, :] * lnf[None, :]).T  # [D, VS]
        eT = eT.reshape(DT, P, NVCH, VCH)
        m["embT"] = np.ascontiguousarray(eT.transpose(2, 1, 0, 3), np.float32)
        in_maps.append(m)
    return in_maps


def kernel(**inputs):
    global _compiled
    if _compiled is None:
        _compiled = build()
    in_maps = _prep_inputs(inputs)
    res = run_bass_kernel_spmd(_compiled, in_maps, core_ids=list(range(CORES)))
    shards = [res.results[c]["out"].reshape(B, S, VS) for c in range(CORES)]
    return np.concatenate(shards, axis=-1)


if __name__ == "__main__":
    import reference as R
    inputs = R.setup_inputs()
    out = kernel(**{k: np.asarray(v) for k, v in inputs.items()})
    print("kernel out", out.shape, out.dtype)
